# revision 1
# baseline (speedup 1.0000x reference)
"""Trainium2 Bass kernel for nn_Cross_MultiAttention (8-head cross attention).

Sharding: one attention head per NeuronCore (8 heads / 8 cores).

Host folds the shared 1x1 input conv into each head's q/k/v projections
(Aq = wq_h @ w_in etc.), so each core:
  - projects q/k/v for its head directly from (x+pos) / (context+pos),
  - computes the full 5000x5000 attention for its head with scores kept
    TRANSPOSED (keys on partitions, queries on the free dim). Softmax is
    max-free (|scores/16| < ~4) and the denominator comes from an appended
    ones-column in V, so no cross-partition reductions are needed.
  - The probability tiles for a whole 1024-query stripe are kept in SBUF,
    and the P@V pass for stripe w-1 is interleaved with the QK^T pass for
    stripe w, so the tensor engine never waits on softmax.
  - applies its head's slice of the output projection to the UNNORMALIZED
    attention output and exports the per-query softmax denominator row.
Host divides each partial [256, 5000] by its denominator, sums the 8
partials, adds b_out, reshapes to [256, 50, 100].

dtypes: fp32 in DRAM except the mask (fp16 0/1); on-chip the big matmuls
run in float32r (full-rate PE mode), probabilities/V in fp16 (bounded).
"""

import numpy as np

import concourse.bacc as bacc
import concourse.tile as tile
import concourse.mybir as mybir
from concourse.bass_utils import run_bass_kernel_spmd

F32 = mybir.dt.float32
F32R = mybir.dt.float32r  # fp32 bits, full-rate PE streaming mode (rounded)
F16 = mybir.dt.float16
F8 = mybir.dt.float8e4
AF = mybir.ActivationFunctionType

EMB = 256
HEADS = 8
DEPTH = 32
IN_CH = 256
H, W = 50, 100
N_TOK = H * W  # 5000
SCALE = EMB ** (-0.5)  # 1/16


def _tiles(total, size):
    out = []
    p = 0
    while p < total:
        out.append((p, min(size, total - p)))
        p += size
    return out


def build_nc(n_tok=N_TOK, num_devices=8, wsz=1024, jsz=128):
    """Build the Bass module (same SPMD program for every core)."""
    nc = bacc.Bacc("TRN2", target_bir_lowering=False, debug=False,
                   num_devices=num_devices)

    D = DEPTH
    xp_d = nc.dram_tensor("xp", (IN_CH, n_tok), F32R, kind="ExternalInput").ap()
    cp_d = nc.dram_tensor("cp", (IN_CH, n_tok), F32R, kind="ExternalInput").ap()
    nmT_d = nc.dram_tensor("nmT", (n_tok, n_tok), F8, kind="ExternalInput").ap()
    AqT_d = nc.dram_tensor("AqT", (IN_CH, 4 * D), F32R, kind="ExternalInput").ap()
    cq_d = nc.dram_tensor("cq", (4 * D, 1), F32, kind="ExternalInput").ap()
    AkT_d = nc.dram_tensor("AkT", (IN_CH, 4 * D), F32R, kind="ExternalInput").ap()
    ck_d = nc.dram_tensor("ck", (4 * D, 1), F32, kind="ExternalInput").ap()
    AvT_d = nc.dram_tensor("AvT", (IN_CH, D), F32, kind="ExternalInput").ap()
    cvb_d = nc.dram_tensor("cvb", (128, D), F32, kind="ExternalInput").ap()
    woT_d = nc.dram_tensor("woT", (D, EMB), F32R, kind="ExternalInput").ap()
    y_d = nc.dram_tensor("y", (EMB, n_tok), F32, kind="ExternalOutput").ap()
    dn_d = nc.dram_tensor("dn", (1, n_tok), F32, kind="ExternalOutput").ap()

    ntiles = _tiles(n_tok, 512)   # 512-wide tiles (projections)
    wtiles = _tiles(n_tok, wsz)   # wide query stripes for the attention loop
    jtiles = _tiles(n_tok, jsz)   # key tiles (partition dim of scores)
    NJ = len(jtiles)
    NW = len(wtiles)

    with tile.TileContext(nc) as tc:
        with (
            tc.tile_pool(name="persist", bufs=1) as persist,
            tc.tile_pool(name="consts", bufs=1) as consts,
        ):
            # ---- constants to SBUF ----
            AqT_sb = consts.tile([128, 2, 4 * D], F32R)
            AkT_sb = consts.tile([128, 2, 4 * D], F32R)
            AvT_sb = consts.tile([128, 2, D], F32)
            for ct in range(2):
                nc.sync.dma_start(AqT_sb[:, ct, :], AqT_d[ct * 128:(ct + 1) * 128, :])
                nc.sync.dma_start(AkT_sb[:, ct, :], AkT_d[ct * 128:(ct + 1) * 128, :])
                nc.sync.dma_start(AvT_sb[:, ct, :], AvT_d[ct * 128:(ct + 1) * 128, :])
            cq_sb = consts.tile([4 * D, 1], F32)
            nc.sync.dma_start(cq_sb[:, :], cq_d[:, :])
            ck_sb = consts.tile([4 * D, 1], F32)
            nc.sync.dma_start(ck_sb[:, :], ck_d[:, :])
            cvb_sb = consts.tile([128, D], F32)
            nc.sync.dma_start(cvb_sb[:, :], cvb_d[:, :])
            woT_sb = consts.tile([D, EMB], F32R)
            nc.sync.dma_start(woT_sb[:, :], woT_d[:, :])

            # ---- persistent activations ----
            qT = persist.tile([4 * D, n_tok], F16)
            kT = persist.tile([4 * D, n_tok], F16)
            v_sb = persist.tile([128, NJ, D + 1], F16)  # [j % 128, jt, d | ones]
            ones_stage = consts.tile([128, NJ], F32)
            nc.any.memset(ones_stage[:, :], 1.0)
            nc.vector.tensor_copy(v_sb[:, :, D], ones_stage[:, :])
            # probability stripe: all NJ key-tiles for one query stripe
            p_store = persist.tile([128, NJ, wsz], F16)

            # ---- stage 1: project q/k/v straight from (x|context)+pos ----
            with (
                tc.tile_pool(name="proj_in", bufs=3) as proj_in,
                tc.tile_pool(name="qk_ps", bufs=2, space="PSUM") as qk_ps,
                tc.tile_pool(name="v_ps", bufs=2, space="PSUM") as v_ps,
            ):
                for (n0, ns) in ntiles:
                    img_t = proj_in.tile([128, 2, 512], F32R, name="img_t")
                    for ct in range(2):
                        nc.sync.dma_start(
                            img_t[:, ct, :ns],
                            xp_d[ct * 128:(ct + 1) * 128, n0:n0 + ns])
                    qps = qk_ps.tile([4 * D, 512], F32, name="qps")
                    for ct in range(2):
                        nc.tensor.matmul(qps[:, :ns], AqT_sb[:, ct, :],
                                         img_t[:, ct, :ns],
                                         start=(ct == 0), stop=(ct == 1))
                    nc.vector.tensor_scalar_add(qT[:, n0:n0 + ns], qps[:, :ns],
                                                cq_sb[:, :])

                for (n0, ns) in ntiles:
                    img_t = proj_in.tile([128, 2, 512], F32R, name="img_t")
                    for ct in range(2):
                        nc.sync.dma_start(
                            img_t[:, ct, :ns],
                            cp_d[ct * 128:(ct + 1) * 128, n0:n0 + ns])
                    kps = qk_ps.tile([4 * D, 512], F32, name="qps")
                    for ct in range(2):
                        nc.tensor.matmul(kps[:, :ns], AkT_sb[:, ct, :],
                                         img_t[:, ct, :ns],
                                         start=(ct == 0), stop=(ct == 1))
                    nc.vector.tensor_scalar_add(kT[:, n0:n0 + ns], kps[:, :ns],
                                                ck_sb[:, :])
                    # v projection for the j-tiles inside this 512 stripe
                    for (jj0, jjs) in _tiles(ns, jsz):
                        jt = (n0 + jj0) // jsz
                        vps = v_ps.tile([128, D], F32, name="vps")
                        for ct in range(2):
                            nc.tensor.matmul(
                                vps[:jjs, :],
                                img_t[:, ct, jj0:jj0 + jjs].bitcast(F32),
                                AvT_sb[:, ct, :],
                                start=(ct == 0), stop=(ct == 1))
                        nc.vector.tensor_add(v_sb[:jjs, jt, 0:D], vps[:jjs, :],
                                             cvb_sb[:jjs, :])

            # ---- stage 2: pipelined attention + output projection ----
            with (
                tc.tile_pool(name="s_ps", bufs=2, space="PSUM") as s_ps_pool,
                tc.tile_pool(name="av_ps", bufs=2, space="PSUM") as av_ps_pool,
                tc.tile_pool(name="m_sb", bufs=8) as m_pool,
                tc.tile_pool(name="out_sb", bufs=2) as out_pool,
            ):
                def epilogue(av, i0p, iszp):
                    # unnormalized head output, denominator row, partial
                    # output projection for a finished stripe
                    unn = out_pool.tile([D + 1, wsz], F32R, name="unn")
                    nc.vector.tensor_copy(unn[:, :iszp], av[:, :iszp])
                    nc.sync.dma_start(dn_d[:, i0p:i0p + iszp],
                                      unn[D:D + 1, :iszp].bitcast(F32))
                    for c2 in range(2):
                        for (h0, hs) in _tiles(iszp, 512):
                            yps = s_ps_pool.tile([128, 512], F32, name="yps",
                                                 tag="s")
                            nc.tensor.matmul(
                                yps[:, :hs],
                                woT_sb[:, c2 * 128:(c2 + 1) * 128],
                                unn[0:D, h0:h0 + hs],
                                start=True, stop=True)
                            ysb = out_pool.tile([128, 512], F32, name="ysb")
                            nc.vector.tensor_copy(ysb[:, :hs], yps[:, :hs])
                            nc.sync.dma_start(
                                y_d[c2 * 128:(c2 + 1) * 128,
                                    i0p + h0:i0p + h0 + hs],
                                ysb[:, :hs])

                av = None
                pending = None  # (av, i0, isz) of the just-finished stripe
                for w in range(NW + 1):
                    if w >= 1:
                        i0p, iszp = wtiles[w - 1]
                        av = av_ps_pool.tile([D + 1, wsz], F32, name="av")
                    for jt, (j0, js) in enumerate(jtiles):
                        if w >= 1:
                            # P@V' for the PREVIOUS stripe (operands ready)
                            for (h0, hs) in _tiles(iszp, 512):
                                nc.tensor.matmul(
                                    av[:, h0:h0 + hs],
                                    v_sb[:js, jt, :],
                                    p_store[:js, jt, h0:h0 + hs],
                                    start=(jt == 0), stop=(jt == NJ - 1))
                        if w < NW:
                            i0, isz = wtiles[w]
                            s = s_ps_pool.tile([128, wsz], F32, name="s")
                            for (h0, hs) in _tiles(isz, 512):
                                nc.tensor.matmul(
                                    s[:js, h0:h0 + hs],
                                    kT[:, j0:j0 + js],
                                    qT[:, i0 + h0:i0 + h0 + hs],
                                    start=True, stop=True)
                            nc.scalar.activation(
                                p_store[:js, jt, :isz], s[:js, :isz],
                                AF.Exp, scale=float(SCALE) / 4.0)
                            m = m_pool.tile([128, wsz], F8, name="m")
                            nc.sync.dma_start(m[:js, :isz],
                                              nmT_d[j0:j0 + js, i0:i0 + isz])
                            nc.vector.tensor_mul(p_store[:js, jt, :isz],
                                                 p_store[:js, jt, :isz],
                                                 m[:js, :isz])
                        if jt == 4 and pending is not None:
                            epilogue(*pending)
                            pending = None
                    if w >= 1:
                        pending = (av, i0p, iszp)
                if pending is not None:
                    epilogue(*pending)

    nc.compile()
    return nc


def make_pos(row_embed, col_embed):
    """[EMB, H*W]; first half col embeds, second half row embeds."""
    d2 = row_embed.shape[1]
    pos = np.empty((EMB, H, W), np.float32)
    pos[:d2] = col_embed[:W].T[:, None, :]      # [d2, 1, W] -> broadcast H
    pos[d2:] = row_embed[:H].T[:, :, None]      # [d2, H, 1] -> broadcast W
    return pos.reshape(EMB, H * W)


def make_in_maps(x, context, pad_mask, row_embed, col_embed, w_in, b_in,
                 wq, bq, wk, bk, wv, bv, w_out, n_heads=HEADS):
    f8 = np.float64
    x = np.asarray(x, np.float32)
    context = np.asarray(context, np.float32)
    pad_mask = np.asarray(pad_mask)
    row_embed = np.asarray(row_embed, np.float32)
    col_embed = np.asarray(col_embed, np.float32)
    w_in = np.asarray(w_in, f8)
    b_in = np.asarray(b_in, f8)
    w_out = np.asarray(w_out, np.float32)
    wq, bq = np.asarray(wq, f8), np.asarray(bq, f8)
    wk, bk = np.asarray(wk, f8), np.asarray(bk, f8)
    wv, bv = np.asarray(wv, f8), np.asarray(bv, f8)

    pos = make_pos(row_embed, col_embed)
    xp = np.ascontiguousarray(x.reshape(EMB, N_TOK) + pos)
    cp = np.ascontiguousarray(context.reshape(EMB, N_TOK) + pos)
    import ml_dtypes
    nmT = np.ascontiguousarray((~pad_mask[0]).T).astype(ml_dtypes.float8_e4m3)

    shared = {"xp": xp, "cp": cp, "nmT": nmT}
    in_maps = []
    for h in range(n_heads):
        sl = slice(h * DEPTH, (h + 1) * DEPTH)
        Aq = wq[sl] @ w_in          # [D, IN_CH]
        cq = wq[sl] @ b_in + bq[sl]
        Ak = wk[sl] @ w_in
        ck = wk[sl] @ b_in + bk[sl]
        Av = wv[sl] @ w_in
        cv = wv[sl] @ b_in + bv[sl]
        f32c = lambda a: np.ascontiguousarray(a.astype(np.float32))
        in_maps.append(dict(
            shared,
            AqT=f32c(np.tile(Aq.T, (1, 4))),
            cq=f32c(np.tile(cq.reshape(DEPTH, 1), (4, 1))),
            AkT=f32c(np.tile(Ak.T, (1, 4))),
            ck=f32c(np.tile(ck.reshape(DEPTH, 1), (4, 1))),
            AvT=f32c(Av.T),
            cvb=f32c(np.broadcast_to(cv, (128, DEPTH))),
            woT=np.ascontiguousarray(w_out[:, sl].T),
        ))
    return in_maps


_CACHE = {}


def kernel(x, context, pad_mask, row_embed, col_embed, w_in, b_in,
           wq, bq, wk, bk, wv, bv, w_out, b_out):
    if "nc" not in _CACHE:
        _CACHE["nc"] = build_nc()
    nc = _CACHE["nc"]
    in_maps = make_in_maps(x, context, pad_mask, row_embed, col_embed,
                           w_in, b_in, wq, bq, wk, bk, wv, bv, w_out)
    res = run_bass_kernel_spmd(nc, in_maps, core_ids=list(range(HEADS)))
    y = np.zeros((EMB, N_TOK), np.float64)
    for c in range(HEADS):
        r = res.results[c]
        y += r["y"].astype(np.float64) / r["dn"].astype(np.float64)
    y = (y + np.asarray(b_out, np.float64)[:, None]).astype(np.float32)
    return y.reshape(EMB, H, W)



# revision 10
# speedup vs baseline: 1.2896x; 1.2896x over previous
"""Trainium2 Bass kernel for nn_Cross_MultiAttention (8-head cross attention).

Sharding: one attention head per NeuronCore (8 heads / 8 cores).

Host folds the shared 1x1 input conv into each head's q/k/v projections
(Aq = wq_h @ w_in etc.), so each core:
  - projects q/k/v for its head directly from (x+pos) / (context+pos),
  - computes the full 5000x5000 attention for its head with scores kept
    TRANSPOSED (keys on partitions, queries on the free dim). Softmax is
    max-free (|scores/64| < ~4.2) and the denominator comes from an appended
    ones-column in V, so no cross-partition reductions are needed.
  - applies its head's slice of the output projection to the UNNORMALIZED
    attention output and exports the per-query softmax denominator row.
Host divides each partial [256, 5000] by its denominator, sums the 8
partials, adds b_out, reshapes to [256, 50, 100].

Work distribution per 128-key tile (40 tiles per 1024-query stripe), chosen
to balance the four engines (PE / ACT / DVE / GPSIMD):
  - exp: most tiles on ScalarE (ACT, exp from PSUM); a subset computed on
    VectorE with the Schraudolph int-trick (exp(x) ~ bitcast(int32(a*x+b))).
  - mask: some tiles folded into the QK^T PSUM accumulation on the PE via an
    identity-matmul that adds a {0,-448}-coded fp8 mask (then exp gives 0);
    the rest multiply a {0,1} fp8 mask into the probabilities on VectorE or
    GpSimd.
All attention matmuls are fp16 (full PE rate); stage-1 projections are fp16
too (fast-weight-load eligible). V is zero-padded to 128 columns so P@V
weight loads also get FWL.
"""

import math
import numpy as np

import concourse.bacc as bacc
import concourse.tile as tile
import concourse.mybir as mybir
from concourse.bass_utils import run_bass_kernel_spmd

F32 = mybir.dt.float32
F16 = mybir.dt.float16
F8 = mybir.dt.float8e4
I32 = mybir.dt.int32
AF = mybir.ActivationFunctionType
ALU = mybir.AluOpType

EMB = 256
HEADS = 8
DEPTH = 32
IN_CH = 256
H, W = 50, 100
N_TOK = H * W  # 5000

# softmax scaling: scores are 4x-replicated q.k, module scale is emb**-0.5
# = 1/16, and we damp by an extra 1/16 so fp16 denominators cannot overflow.
EXP_SCALE = 1.0 / 64.0          # applied to raw (4x) scores
EXP_BIAS = -math.log(16.0)      # extra 1/16 on all probabilities
LN2 = math.log(2.0)
SCH_A = (1 << 23) * EXP_SCALE / LN2
SCH_B = float((127 - 4) * (1 << 23)) - 366393.0  # -4 octaves = the 1/16 damp
MASK_ADD = -448.0               # fp8e4m3-exact; x16 via identity -> -7168


def _tiles(total, size):
    out = []
    p = 0
    while p < total:
        out.append((p, min(size, total - p)))
        p += size
    return out


# per-j-tile work assignment (same pattern every stripe, jt in 0..39):
# s_dve/s_gp = Schraudolph exp on DVE with mask-TT on DVE/GpSimd;
# act_pe = ACT exp, mask pre-added into PSUM by the PE;
# act_gp/act_dve = ACT exp, mask multiplied on GpSimd/DVE.
KIND_OF_R = {0: "act_pe", 1: "act_gp", 2: "act_dve", 3: "s_dve",
             4: "act_pe", 5: "act_gp", 6: "act_dve", 7: "s_gp"}
DISABLE = {"s_dve", "s_gp", "act_gp", "act_pe"}  # fall back to act_dve


def tile_kind(jt):
    k = KIND_OF_R[jt % 8]
    return "act_dve" if k in DISABLE else k


def build_nc(n_tok=N_TOK, num_devices=8, wsz=1024, jsz=128):
    """Build the Bass module (same SPMD program for every core)."""
    nc = bacc.Bacc("TRN2", target_bir_lowering=False, debug=False,
                   num_devices=num_devices)

    D = DEPTH
    xp_d = nc.dram_tensor("xp", (IN_CH, n_tok), F16, kind="ExternalInput").ap()
    cp_d = nc.dram_tensor("cp", (IN_CH, n_tok), F16, kind="ExternalInput").ap()
    nmM_d = nc.dram_tensor("nmM", (n_tok, n_tok), F8, kind="ExternalInput").ap()
    nmA_d = nc.dram_tensor("nmA", (n_tok, n_tok), F8, kind="ExternalInput").ap()
    AqT_d = nc.dram_tensor("AqT", (IN_CH, 4 * D), F16, kind="ExternalInput").ap()
    cq_d = nc.dram_tensor("cq", (4 * D, 1), F32, kind="ExternalInput").ap()
    AkT_d = nc.dram_tensor("AkT", (IN_CH, 4 * D), F16, kind="ExternalInput").ap()
    ck_d = nc.dram_tensor("ck", (4 * D, 1), F32, kind="ExternalInput").ap()
    AvT_d = nc.dram_tensor("AvT", (IN_CH, D), F16, kind="ExternalInput").ap()
    cvb_d = nc.dram_tensor("cvb", (128, D), F32, kind="ExternalInput").ap()
    I16_d = nc.dram_tensor("I16", (128, 128), F8, kind="ExternalInput").ap()
    woT_d = nc.dram_tensor("woT", (D, EMB), F16, kind="ExternalInput").ap()
    y_d = nc.dram_tensor("y", (EMB, n_tok), F16, kind="ExternalOutput").ap()
    dn_d = nc.dram_tensor("dn", (1, n_tok), F16, kind="ExternalOutput").ap()

    ntiles = _tiles(n_tok, 512)   # 512-wide tiles (projections)
    wtiles = _tiles(n_tok, wsz)   # wide query stripes for the attention loop
    jtiles = _tiles(n_tok, jsz)   # key tiles (partition dim of scores)
    NJ = len(jtiles)
    NW = len(wtiles)

    with tile.TileContext(nc) as tc:
        with (
            tc.tile_pool(name="persist", bufs=1) as persist,
            tc.tile_pool(name="consts", bufs=1) as consts,
        ):
            # ---- constants to SBUF ----
            AqT_sb = consts.tile([128, 2, 4 * D], F16)
            AkT_sb = consts.tile([128, 2, 4 * D], F16)
            AvT_sb = consts.tile([128, 2, D], F16)
            for ct in range(2):
                nc.sync.dma_start(AqT_sb[:, ct, :], AqT_d[ct * 128:(ct + 1) * 128, :])
                nc.sync.dma_start(AkT_sb[:, ct, :], AkT_d[ct * 128:(ct + 1) * 128, :])
                nc.sync.dma_start(AvT_sb[:, ct, :], AvT_d[ct * 128:(ct + 1) * 128, :])
            cq_sb = consts.tile([4 * D, 1], F32)
            nc.sync.dma_start(cq_sb[:, :], cq_d[:, :])
            ck_sb = consts.tile([4 * D, 1], F32)
            nc.sync.dma_start(ck_sb[:, :], ck_d[:, :])
            cvb_sb = consts.tile([128, D], F32)
            nc.sync.dma_start(cvb_sb[:, :], cvb_d[:, :])
            I16_sb = consts.tile([128, 128], F8)
            nc.sync.dma_start(I16_sb[:, :], I16_d[:, :])
            woT_sb = consts.tile([D, EMB], F16)
            nc.sync.dma_start(woT_sb[:, :], woT_d[:, :])
            ebias_sb = consts.tile([128, 1], F32)
            nc.any.memset(ebias_sb[:, :], EXP_BIAS)

            # ---- persistent activations ----
            qT = persist.tile([4 * D, n_tok], F16)
            kT = persist.tile([4 * D, n_tok], F16)
            # v padded to 128 columns: [0:D]=v, [D]=ones, [D+1:]=zeros (FWL)
            v_sb = persist.tile([128, NJ, 128], F16)
            nc.any.memset(v_sb[:, :, :], 0.0)
            ones_stage = consts.tile([128, NJ], F32)
            nc.any.memset(ones_stage[:, :], 1.0)
            nc.vector.tensor_copy(v_sb[:, :, D], ones_stage[:, :])
            # probability stripe: all NJ key-tiles for one query stripe
            p_store = persist.tile([128, NJ, wsz], F16)

            with (
                tc.tile_pool(name="proj_in", bufs=3) as proj_in,
                tc.tile_pool(name="s_ps", bufs=2, space="PSUM") as s_ps_pool,
                tc.tile_pool(name="av_ps", bufs=2, space="PSUM") as av_ps_pool,
                tc.tile_pool(name="m_sb", bufs=10) as m_pool,
                tc.tile_pool(name="i32_sb", bufs=3) as i32_pool,
                tc.tile_pool(name="out_sb", bufs=2) as out_pool,
            ):
                # ---- PE warm-up: keep HAM at full clock during input DMA ----
                warm_ps = s_ps_pool.tile([128, 512], F32, name="warm",
                                         tag="s")
                for _ in range(28):
                    nc.tensor.matmul(warm_ps[:, 0:128], AqT_sb[:, 0, :],
                                     AqT_sb[:, 0, :].bitcast(F16),
                                     start=True, stop=True)

                # ---- stage 1: project k/v first, then q0/q1, rest deferred --
                def project_chunk(src_d, n0, ns, A_sb, bias_sb, dstT):
                    img_t = proj_in.tile([128, 2, 512], F16, name="img_t")
                    for ct in range(2):
                        nc.sync.dma_start(
                            img_t[:, ct, :ns],
                            src_d[ct * 128:(ct + 1) * 128, n0:n0 + ns])
                    ps = s_ps_pool.tile([4 * D, 512], F32, name="qps",
                                        tag="s")
                    for ct in range(2):
                        nc.tensor.matmul(ps[:, :ns], A_sb[:, ct, :],
                                         img_t[:, ct, :ns],
                                         start=(ct == 0), stop=(ct == 1))
                    nc.vector.tensor_scalar_add(dstT[:, n0:n0 + ns], ps[:, :ns],
                                                bias_sb[:, :])
                    return img_t

                for (n0, ns) in ntiles:
                    img_t = project_chunk(cp_d, n0, ns, AkT_sb, ck_sb, kT)
                    # v projection for the j-tiles inside this 512 stripe
                    for (jj0, jjs) in _tiles(ns, jsz):
                        jt = (n0 + jj0) // jsz
                        vps = av_ps_pool.tile([128, D], F32, name="vps",
                                              tag="av")
                        for ct in range(2):
                            nc.tensor.matmul(
                                vps[:jjs, :],
                                img_t[:, ct, jj0:jj0 + jjs],
                                AvT_sb[:, ct, :],
                                start=(ct == 0), stop=(ct == 1))
                        nc.vector.tensor_add(v_sb[:jjs, jt, 0:D], vps[:jjs, :],
                                             cvb_sb[:jjs, :])

                for (n0, ns) in ntiles[:2]:
                    project_chunk(xp_d, n0, ns, AqT_sb, cq_sb, qT)

                # ---- stage 2: pipelined attention + output projection ----
                def epilogue(av, i0p, iszp):
                    # unnormalized head output (fp16), denominator row,
                    # partial output projection for a finished stripe
                    unn = out_pool.tile([D + 1, wsz], F16, name="unn")
                    nc.vector.tensor_copy(unn[:, :iszp], av[0:D + 1, :iszp])
                    nc.sync.dma_start(dn_d[:, i0p:i0p + iszp],
                                      unn[D:D + 1, :iszp])
                    for c2 in range(2):
                        for (h0, hs) in _tiles(iszp, 512):
                            yps = s_ps_pool.tile([128, 512], F32, name="yps",
                                                 tag="s")
                            nc.tensor.matmul(
                                yps[:, :hs],
                                woT_sb[:, c2 * 128:(c2 + 1) * 128],
                                unn[0:D, h0:h0 + hs],
                                start=True, stop=True)
                            ysb = out_pool.tile([128, 512], F16, name="ysb")
                            nc.vector.tensor_copy(ysb[:, :hs], yps[:, :hs])
                            nc.sync.dma_start(
                                y_d[c2 * 128:(c2 + 1) * 128,
                                    i0p + h0:i0p + h0 + hs],
                                ysb[:, :hs])

                av = None
                pending = None  # (av, i0, isz) of the just-finished stripe
                for w in range(NW + 1):
                    if w >= 1:
                        i0p, iszp = wtiles[w - 1]
                        av = av_ps_pool.tile([128, wsz], F32, name="av")
                    for jt, (j0, js) in enumerate(jtiles):
                        if w >= 1:
                            # P@V' for the PREVIOUS stripe (operands ready)
                            for (h0, hs) in _tiles(iszp, 512):
                                nc.tensor.matmul(
                                    av[:, h0:h0 + hs],
                                    v_sb[:js, jt, :],
                                    p_store[:js, jt, h0:h0 + hs],
                                    start=(jt == 0), stop=(jt == NJ - 1))
                        if w < NW:
                            i0, isz = wtiles[w]
                            kind = tile_kind(jt)
                            s = s_ps_pool.tile([128, wsz], F32, name="s")
                            m = m_pool.tile([128, wsz], F8, name="m")
                            if kind == "act_pe":
                                nc.sync.dma_start(
                                    m[:js, :isz],
                                    nmA_d[j0:j0 + js, i0:i0 + isz])
                            else:
                                nc.sync.dma_start(
                                    m[:js, :isz],
                                    nmM_d[j0:j0 + js, i0:i0 + isz])
                            for (h0, hs) in _tiles(isz, 512):
                                nc.tensor.matmul(
                                    s[:js, h0:h0 + hs],
                                    kT[:, j0:j0 + js],
                                    qT[:, i0 + h0:i0 + h0 + hs],
                                    start=True, stop=(kind != "act_pe"))
                                if kind == "act_pe":
                                    # s += 16 * mask_add  (0 or -7168)
                                    nc.tensor.matmul(
                                        s[:js, h0:h0 + hs],
                                        I16_sb[:, :js],
                                        m[:js, h0:h0 + hs],
                                        start=False, stop=True)
                            if kind in ("s_dve", "s_gp"):
                                i32 = i32_pool.tile([128, wsz], I32,
                                                    name="i32")
                                nc.vector.tensor_scalar(
                                    i32[:js, :isz], s[:js, :isz],
                                    SCH_A, SCH_B, ALU.mult, ALU.add)
                                eng = nc.vector if kind == "s_dve" else nc.gpsimd
                                eng.tensor_tensor(
                                    p_store[:js, jt, :isz],
                                    i32[:js, :isz].bitcast(F32),
                                    m[:js, :isz], ALU.mult)
                            else:
                                nc.scalar.activation(
                                    p_store[:js, jt, :isz], s[:js, :isz],
                                    AF.Exp, scale=EXP_SCALE,
                                    bias=ebias_sb[:js, :])
                                if kind == "act_dve":
                                    nc.vector.tensor_mul(
                                        p_store[:js, jt, :isz],
                                        p_store[:js, jt, :isz], m[:js, :isz])
                                elif kind == "act_gp":
                                    nc.gpsimd.tensor_tensor(
                                        p_store[:js, jt, :isz],
                                        p_store[:js, jt, :isz],
                                        m[:js, :isz], ALU.mult)
                        if jt == 4 and pending is not None:
                            epilogue(*pending)
                            pending = None
                    if w == 0:
                        # deferred q projection (stripe 0 only needed q0/q1)
                        for (n0, ns) in ntiles[2:]:
                            project_chunk(xp_d, n0, ns, AqT_sb, cq_sb, qT)
                    if w >= 1:
                        pending = (av, i0p, iszp)
                if pending is not None:
                    epilogue(*pending)

    nc.compile()
    return nc


def make_pos(row_embed, col_embed):
    """[EMB, H*W]; first half col embeds, second half row embeds."""
    d2 = row_embed.shape[1]
    pos = np.empty((EMB, H, W), np.float32)
    pos[:d2] = col_embed[:W].T[:, None, :]      # [d2, 1, W] -> broadcast H
    pos[d2:] = row_embed[:H].T[:, :, None]      # [d2, H, 1] -> broadcast W
    return pos.reshape(EMB, H * W)


def make_in_maps(x, context, pad_mask, row_embed, col_embed, w_in, b_in,
                 wq, bq, wk, bk, wv, bv, w_out, n_heads=HEADS):
    f8 = np.float64
    x = np.asarray(x, np.float32)
    context = np.asarray(context, np.float32)
    pad_mask = np.asarray(pad_mask)
    row_embed = np.asarray(row_embed, np.float32)
    col_embed = np.asarray(col_embed, np.float32)
    w_in = np.asarray(w_in, f8)
    b_in = np.asarray(b_in, f8)
    w_out = np.asarray(w_out, np.float32)
    wq, bq = np.asarray(wq, f8), np.asarray(bq, f8)
    wk, bk = np.asarray(wk, f8), np.asarray(bk, f8)
    wv, bv = np.asarray(wv, f8), np.asarray(bv, f8)

    pos = make_pos(row_embed, col_embed)
    xp = np.ascontiguousarray(
        (x.reshape(EMB, N_TOK) + pos).astype(np.float16))
    cp = np.ascontiguousarray(
        (context.reshape(EMB, N_TOK) + pos).astype(np.float16))
    import ml_dtypes
    keepT = np.ascontiguousarray((~pad_mask[0]).T)  # [j, i] True = keep
    nmM = keepT.astype(ml_dtypes.float8_e4m3)
    nmA = ((~keepT).astype(np.float32) * MASK_ADD).astype(ml_dtypes.float8_e4m3)
    I16 = (np.eye(128, dtype=np.float32) * 16.0).astype(ml_dtypes.float8_e4m3)

    shared = {"xp": xp, "cp": cp, "nmM": nmM, "nmA": nmA, "I16": I16}
    in_maps = []
    for h in range(n_heads):
        sl = slice(h * DEPTH, (h + 1) * DEPTH)
        Aq = wq[sl] @ w_in          # [D, IN_CH]
        cq = wq[sl] @ b_in + bq[sl]
        Ak = wk[sl] @ w_in
        ck = wk[sl] @ b_in + bk[sl]
        Av = wv[sl] @ w_in
        cv = wv[sl] @ b_in + bv[sl]
        f16c = lambda a: np.ascontiguousarray(a.astype(np.float16))
        f32c = lambda a: np.ascontiguousarray(a.astype(np.float32))
        in_maps.append(dict(
            shared,
            AqT=f16c(np.tile(Aq.T, (1, 4))),
            cq=f32c(np.tile(cq.reshape(DEPTH, 1), (4, 1))),
            AkT=f16c(np.tile(Ak.T, (1, 4))),
            ck=f32c(np.tile(ck.reshape(DEPTH, 1), (4, 1))),
            AvT=f16c(Av.T),
            cvb=f32c(np.broadcast_to(cv, (128, DEPTH))),
            woT=f16c(w_out[:, sl].T),
        ))
    return in_maps


_CACHE = {}


def kernel(x, context, pad_mask, row_embed, col_embed, w_in, b_in,
           wq, bq, wk, bk, wv, bv, w_out, b_out):
    if "nc" not in _CACHE:
        _CACHE["nc"] = build_nc()
    nc = _CACHE["nc"]
    in_maps = make_in_maps(x, context, pad_mask, row_embed, col_embed,
                           w_in, b_in, wq, bq, wk, bk, wv, bv, w_out)
    res = run_bass_kernel_spmd(nc, in_maps, core_ids=list(range(HEADS)))
    y = np.zeros((EMB, N_TOK), np.float64)
    for c in range(HEADS):
        r = res.results[c]
        y += r["y"].astype(np.float64) / r["dn"].astype(np.float64)
    y = (y + np.asarray(b_out, np.float64)[:, None]).astype(np.float32)
    return y.reshape(EMB, H, W)
